# revision 21
# baseline (speedup 1.0000x reference)
"""Trainium2 Bass kernel for nn_CombinedGNN (gnn_message_passing).

Strategy (8 NeuronCores, node/row parallel, zero collectives):
  - masks[1] in the reference is identically zero (elementwise pow of a 0/1
    matrix), so only mask0 = adj/rowdeg matters.
  - All T=12 timesteps' aggregations are mask0 @ data[t] -> batched into ONE
    accumulated matmul  X^T @ adjT  with X = data rearranged to [N, 96].
    The row 1/deg scaling uses a host-precomputed inverse-degree vector
    broadcast to 96 partitions via a tiny matmul.
  - adj is exactly representable in fp8e4 (0/1): the big matmul runs in
    fp8 DoubleRow perf mode. X is quantized to fp8e4 hi-only (measured
    end-to-end rel err ~6e-3, well under the 2e-2 gate).
  - Per-pair (256 contraction rows) DMA granularity on both HWDGE rings
    with a greedy rate-balanced schedule so phase 1 chases the stream.
  - Dummy warm-up matmuls run from kernel start so the PE HAM clock-gate
    unthrottles (1.2 -> 2.4 GHz) before phase 1, and PE-filler work keeps
    it warm through the chain.
  - The sequential t-chain runs [feature-on-partition, node-on-free].
    Serial path per t is only: relu(prevacc) -> prev-matmul -> relu(p1)
    -> pcomb-matmul.  The raw/agg partial matmuls (batched 4 timesteps
    per 32-aligned PSUM window, PE operands must be 32-aligned) and the
    pos contribution (folded into the pcomb accumulation as extra
    matmuls: W@(relu(p1)+pos) = W@relu(p1) + W@pos) are off-path.
    Per-t prev matmuls write their 4-t window with zero-padded weight
    columns so the PSUM out stays 32-aligned.
"""

import numpy as np
import ml_dtypes

import concourse.bass as bass
import concourse.mybir as mybir
import concourse.bass_utils as bass_utils
from concourse.tile import TileContext
from concourse.vector_clock import ScopedClock
from contextlib import contextmanager


@contextmanager
def _lean_drain():
    """Skip end-of-kernel semaphore clears (one-shot NEFF; every
    run_bass_kernel_spmd call reloads the NEFF, which re-zeros sems)."""
    orig = TileContext._drain_and_barrier

    def patched(self, tick_clock, wait_clock):
        nc = self.nc
        drain_inst = nc.sync.drain()
        wait_clock.add_sem_waits(
            drain_inst.ins, ScopedClock({None: tick_clock.global_clock}))
        popped = nc._tile_sem_poison_stack.pop()
        assert popped is self._sem_poison

    TileContext._drain_and_barrier = patched
    try:
        yield
    finally:
        TileContext._drain_and_barrier = orig

# problem constants (hardcoded per harness contract)
N, T, DAY, L = 5000, 12, 8, 2
F = DAY - 1
DIM = T * DAY  # 96
NCORES = 8
NPC = N // NCORES        # 625 nodes per core
NP = 640                 # padded nodes per core
NH = NP // 2             # 320, node half processed per psum chunk
KT = 128                 # contraction tile (partitions; K padded to 5120)
NK = 5120                # padded contraction size
NKT = NK // KT           # 40 k-tiles
NKP = NKT // 2           # 20 DoubleRow k-pairs
XW = DIM                 # 96
NB = 3                   # 4-timestep blocks (32 p1 rows each)

F32 = mybir.dt.float32
BF16 = mybir.dt.bfloat16
FP8 = mybir.dt.float8e4
BF16_NP = ml_dtypes.bfloat16
FP8_NP = ml_dtypes.float8_e4m3
DR = mybir.MatmulPerfMode.DoubleRow
RELU = mybir.ActivationFunctionType.Relu

_MAXW = 1


def split_multi_waits(nc):
    """Walrus in this container rejects instructions with >~2 sync waits.
    Hoist extra waits onto preceding single-wait NoOps on the same engine."""
    f = nc.m.functions[0]
    for bb in list(f.blocks):
        new, ctr = [], 0
        for inst in bb.instructions:
            si = inst.sync_info
            waits = list(si.on_wait) if (si and si.on_wait) else []
            if len(waits) > _MAXW:
                head, keep = waits[:-_MAXW], waits[-_MAXW:]
                for i in range(0, len(head), _MAXW):
                    nop = mybir.InstNoOp(
                        name=f"{inst.name}-wsplit{ctr}", engine=inst.engine,
                        ins=[], outs=[],
                        sync_info=mybir.SyncInfo(on_wait=head[i:i + _MAXW],
                                                 on_update=[]),
                    )
                    ctr += 1
                    new.append(nop)
                inst.sync_info = mybir.SyncInfo(
                    on_wait=keep,
                    on_update=list(si.on_update) if si.on_update else [])
            new.append(inst)
        bb.instructions = new


def build_nc():
    with _lean_drain():
        return _build_nc_inner()


def _build_nc_inner():
    nc = bass.Bass()
    # a[p, g, i, n] = adjT[(2g+i)*KT + p, n]  (fp8, 0/1 exact)
    a_d = nc.dram_tensor("a", [KT, NKP, 2, NP], FP8, kind="ExternalInput")
    # xh[p, g, i, c] = Xhi[(2g+i)*KT + p, c]
    xh_d = nc.dram_tensor("xh", [KT, NKP, 2, XW], FP8, kind="ExternalInput")
    rbi_d = nc.dram_tensor("rbi", [1, NP], BF16, kind="ExternalInput")
    # wbig96 [96, 832]: cols 0:96 w1r96, 96:192 w1a96, 192:832 dt96
    # (one DMA; block-diagonal raw/agg weights + dt96[8t+f, n] staging)
    wbig96_d = nc.dram_tensor("wbig96", [DIM, 832], BF16,
                              kind="ExternalInput")
    # pt[f, t, n] = pos[t, n, f]
    pt_d = nc.dram_tensor("pt", [8, T, NP], BF16, kind="ExternalInput")
    # w1p96[p, 96t + 8t+o] = prev weight (t) at cols 8t:8t+8, rest zero
    w1p96_d = nc.dram_tensor("w1p96", [8, T * DIM], BF16,
                             kind="ExternalInput")
    # wcombm [40, T, 104]: rows 0:32 = wcomb weights at 8*(t%4)+d (the
    # relu'd 32-row h2 window; zero rows mask the window's incomplete
    # sibling rows), rows 32:40 = the same weights applied to pos(t) (pt
    # is DMA'd into h2 rows 32:40 once) -- pos folds into the SAME matmul
    wcombm_d = nc.dram_tensor("wcombm", [40, T * 104], BF16,
                              kind="ExternalInput")
    out_d = nc.dram_tensor("out", [DIM, NP], BF16, kind="ExternalOutput")

    with TileContext(nc) as tc:
        with (
            tc.tile_pool(name="const", bufs=1) as cpool,
            tc.tile_pool(name="pagg", bufs=2, space="PSUM") as pagg,
            tc.tile_pool(name="prbp1", bufs=2, space="PSUM") as prbp1,
            tc.tile_pool(name="ppc", bufs=2, space="PSUM") as ppc,
            tc.tile_pool(name="pwarm", bufs=1, space="PSUM") as pwarm,
        ):
            # ---------- SBUF tiles ----------
            warm_l = cpool.tile([1, 64], BF16)
            warm_r = cpool.tile([1, 256], BF16)
            ones_t = cpool.tile([1, DIM], BF16)
            rbi_t = cpool.tile([1, NP], BF16)
            xh_t = cpool.tile([KT, NKP, 2, XW], FP8)
            a_t = cpool.tile([KT, NKP, 2, NP], FP8)
            wbig96_t = cpool.tile([DIM, 832], BF16)
            w1p96_t = cpool.tile([8, T * DIM], BF16)
            wcombm_t = cpool.tile([40, T * 104], BF16)
            dagp_t = cpool.tile([8, T, NP], BF16)   # prev staging (t>=1)
            aggs_t = cpool.tile([DIM, NP], BF16)    # scaled agg
            rbs_t = cpool.tile([DIM, NP], BF16)     # inv-degree broadcast
            # rows 0:32 relu(p1) windows, rows 32:40 pos (DMA'd once)
            h2_t = cpool.tile([40, T, NP], BF16)
            outt_t = cpool.tile([DIM, NP], BF16)

            # ---------- DMA schedule ----------
            # Three queues, all carrying critical-stream bytes (the SWDGE
            # ring only helps if its share advances the stream); big
            # transfers leading, small/late ones at the tails.
            nc.gpsimd.dma_start(out=a_t[:, 9:14, :, :],
                                in_=a_d[:, 9:14, :, :])
            nc.sync.dma_start(out=rbi_t, in_=rbi_d[:, :])
            nc.sync.dma_start(out=xh_t[:, 0:10, :, :],
                              in_=xh_d[:, 0:10, :, :])
            nc.sync.dma_start(out=a_t[:, 0:5, :, :], in_=a_d[:, 0:5, :, :])
            nc.scalar.dma_start(out=a_t[:, 5:9, :, :],
                                in_=a_d[:, 5:9, :, :])
            nc.sync.dma_start(out=xh_t[:, 10:NKP, :, :],
                              in_=xh_d[:, 10:NKP, :, :])
            nc.scalar.dma_start(out=a_t[:, 14:NKP, :, :],
                                in_=a_d[:, 14:NKP, :, :])
            nc.sync.dma_start(out=wbig96_t, in_=wbig96_d[:, :])
            nc.sync.dma_start(out=w1p96_t, in_=w1p96_d[:, :])
            nc.scalar.dma_start(out=h2_t[32:40, :, :], in_=pt_d[:, :, :])
            nc.scalar.dma_start(out=wcombm_t, in_=wcombm_d[:, :])

            # ---------- PSUM tiles ----------
            aggp_t = [pagg.tile([XW, NH], F32, tag="aggp", name=f"aggp{h}")
                      for h in range(2)]
            rb_t = [prbp1.tile([DIM, NH], F32, tag="rbp1", name=f"rb{h}")
                    for h in range(2)]
            warmp = pwarm.tile([64, 256], F32, tag="warm", name="warmp")

            # ---------- warm-up / filler matmuls ----------
            nc.vector.memset(warm_l, 0.125)
            nc.vector.memset(warm_r, 0.125)
            nc.vector.memset(ones_t, 1.0)

            def filler(n):
                for _ in range(n):
                    nc.tensor.matmul(warmp, warm_l, warm_r,
                                     start=True, stop=True,
                                     skip_group_check=True)

            filler(12)

            # inv-degree broadcast to 96 partitions, then to SBUF via DVE
            for h in range(2):
                nc.tensor.matmul(rb_t[h], ones_t,
                                 rbi_t[:, h * NH:(h + 1) * NH],
                                 start=True, stop=True, skip_group_check=True)
            for h in range(2):
                nc.vector.tensor_scalar_max(
                    rbs_t[:, h * NH:(h + 1) * NH], rb_t[h], 0.0)

            # p1 tiles (recycle the rb PSUM slots): rows 8t:8t+8 = step t
            p1_t = [prbp1.tile([DIM, NH], F32, tag="rbp1", name=f"p1_{h}")
                    for h in range(2)]
            pcomb_t = [ppc.tile([104, NH], F32, tag="pcm", name=f"pcomb{h}")
                       for h in range(2)]

            # ---------- phase 2 helpers ----------
            def partial_raw(h):
                cs = slice(192 + h * NH, 192 + (h + 1) * NH)
                nc.tensor.matmul(p1_t[h], wbig96_t[:, 0:DIM],
                                 wbig96_t[:, cs],
                                 start=True, stop=False,
                                 skip_group_check=True)

            def partial_agg(h):
                cs = slice(h * NH, (h + 1) * NH)
                nc.tensor.matmul(p1_t[h], wbig96_t[:, DIM:2 * DIM],
                                 aggs_t[:, cs],
                                 start=False, stop=False,
                                 skip_group_check=True)

            def prev_mms(t):
                # adds W_p^T @ prev into rows 8t:8t+8 (weight cols outside
                # the step's 8 rows are zero, so the other 88 rows
                # accumulate +0; keeps all operands at partition base 0)
                for h in range(2):
                    cs = slice(h * NH, (h + 1) * NH)
                    nc.tensor.matmul(p1_t[h],
                                     w1p96_t[:, DIM * t:DIM * (t + 1)],
                                     dagp_t[0:8, t, cs],
                                     start=False,
                                     stop=(t == T - 1),
                                     skip_group_check=True)

            def relu_p1(t):
                # PSUM partition base must be 32-aligned: relu the whole
                # 32-row window (same cost, DVE/ACT time is free-dim-bound);
                # sibling rows may hold partial sums - wcomb4's zero rows
                # mask them in the pcomb matmul.
                b = t // 4
                w = slice(32 * b, 32 * b + 32)
                nc.scalar.activation(h2_t[0:32, t, 0:NH],
                                     p1_t[0][w, :], RELU)
                nc.vector.tensor_scalar_max(h2_t[0:32, t, NH:NP],
                                            p1_t[1][w, :], 0.0)

            def pcomb_mms(t):
                # one matmul adds BOTH wcomb^T relu(p1) and wcomb^T pos(t)
                wc = slice(t * 104, (t + 1) * 104)
                for h in range(2):
                    cs = slice(h * NH, (h + 1) * NH)
                    nc.tensor.matmul(pcomb_t[h], wcombm_t[:, wc],
                                     h2_t[:, t, cs],
                                     start=(t == 0), stop=(t == T - 1),
                                     skip_group_check=True)

            def relu_prev(t):
                # prevacc rows 96:104, mid-group read -> dagp[:, t+1]
                nc.vector.tensor_scalar_max(
                    dagp_t[0:8, t + 1, 0:NH], pcomb_t[0][DIM:104, :], 0.0)
                nc.scalar.activation(dagp_t[0:8, t + 1, NH:NP],
                                     pcomb_t[1][DIM:104, :], RELU)

            # ---------- phase 1: aggT[96, NH] += Xhi^T @ adjT, fp8 DR ------
            # pair order = expected DMA arrival order across the two rings
            # consume a-pairs in DMA-arrival order (the accumulation is
            # order-agnostic): scalar's slices a[5:9]/a[15:20] land
            # ~11-16us while sync's land ~18-22us
            PERM = [5, 6, 7, 8, 0, 1, 2, 3, 4, 9, 10, 11, 12, 13,
                    14, 15, 16, 17, 18, 19]
            for i, g in enumerate(PERM):
                filler(1)
                for h in range(2):
                    nc.tensor.matmul(
                        aggp_t[h], xh_t[:, g, :, :],
                        a_t[:, g, :, h * NH:(h + 1) * NH],
                        start=(i == 0), stop=(i == NKP - 1),
                        perf_mode=DR, skip_group_check=True)

            # ---------- transition: scale by inv-degree ----------
            def scale(h):
                cs = slice(h * NH, (h + 1) * NH)
                nc.vector.tensor_mul(aggs_t[:, cs], aggp_t[h],
                                     rbs_t[:, cs])

            # ---------- phase 2: issue order ----------
            # raw partials + fillers bridge the PE through the scale ops
            # so the HAM clock-gate stays unthrottled into the chain;
            # chain fillers keep PE density high enough to hold it there.
            partial_raw(0)
            partial_raw(1)
            scale(0)
            scale(1)
            partial_agg(0)
            partial_agg(1)
            relu_p1(0)
            pcomb_mms(0)
            relu_prev(0)
            for t in range(1, T):
                prev_mms(t)
                filler(2)
                relu_p1(t)
                pcomb_mms(t)
                if t < T - 1:
                    relu_prev(t)

            # ---------- final relu + store ----------
            nc.scalar.activation(outt_t[:, 0:NH], pcomb_t[0][0:DIM, :], RELU)
            nc.vector.tensor_scalar_max(outt_t[:, NH:NP],
                                        pcomb_t[1][0:DIM, :], 0.0)
            nc.sync.dma_start(out=out_d[:, 0:NH], in_=outt_t[:, 0:NH])
            nc.scalar.dma_start(out=out_d[:, NH:NP], in_=outt_t[:, NH:NP])

    split_multi_waits(nc)
    return nc


def prep_in_maps(adj, data, pos, his_W, cur_W, his_weight, cur_weight,
                 final_weight):
    adj = np.asarray(adj, dtype=np.float32)
    data = np.asarray(data, dtype=np.float32)
    pos = np.asarray(pos, dtype=np.float32)
    his_W = np.asarray(his_W, dtype=np.float32)
    cur_W = np.asarray(cur_W, dtype=np.float32)
    his_weight = np.asarray(his_weight, dtype=np.float32)
    cur_weight = np.asarray(cur_weight, dtype=np.float32)
    final_weight = np.asarray(final_weight, dtype=np.float32)

    # X = data rearranged [N, 96] (col = t*8+d); zero-padded to NK rows;
    # fp8e4 hi quantization (lo pass dropped; error budget allows).
    X = np.ascontiguousarray(data.transpose(1, 0, 2).reshape(N, DIM))
    Xe = np.zeros((NK, XW), np.float32)
    Xe[:N, :] = X
    Xhi = Xe.astype(FP8_NP)
    # DoubleRow lhsT layout: [KT, NKP, 2, XW], [p, g, i, c] = row (2g+i)*KT+p
    def dr_pack(M):
        return np.ascontiguousarray(
            M.reshape(NKP, 2, KT, M.shape[1]).transpose(2, 0, 1, 3))
    xh_h = dr_pack(Xhi)

    adjT = np.ascontiguousarray(adj.T)
    inv_deg = 1.0 / np.maximum(adj.sum(axis=1), 1.0)  # [N]

    # w1 [24, 96]: rows 0:8 prev-block, 8:16 raw, 16:24 agg  (per t cols)
    w1 = np.zeros((24, DIM), np.float32)
    for t in range(T):
        w1[0:7, t * 8:t * 8 + 7] = his_W[t][:, 21:28].T
        w1[7, t * 8 + 7] = cur_W[t][0, 3]
        w1[8:15, t * 8:t * 8 + 7] = his_W[t][:, 0:7].T
        w1[15, t * 8 + 7] = cur_W[t][0, 0]
        w1[16:23, t * 8:t * 8 + 7] = his_W[t][:, 7:14].T
        w1[23, t * 8 + 7] = cur_W[t][0, 1]
    # block-diagonal 96-row lhsT packs (all operands at partition base 0)
    w1r96 = np.zeros((DIM, DIM), np.float32)
    w1a96 = np.zeros((DIM, DIM), np.float32)
    w1p96 = np.zeros((8, T, DIM), np.float32)
    for t in range(T):
        s = slice(8 * t, 8 * t + 8)
        w1r96[s, s] = w1[8:16, s]
        w1a96[s, s] = w1[16:24, s]
        w1p96[:, t, s] = w1[0:8, s]
    w1p96 = np.ascontiguousarray(w1p96.reshape(8, T * DIM))

    # w2s[d, 8t'+o] = prev-update weight from h(t') feature d to output o
    w2 = np.zeros((8, DIM), np.float32)
    for tp in range(T):
        w2[0:7, tp * 8:tp * 8 + 7] = his_weight[:, 7 * tp:7 * tp + 7].T
        w2[7, tp * 8 + 7] = cur_weight[0, tp]
    # interleaved feature (8t+d) -> reference feature (7t+d | 84+t)
    f_ref = np.array([7 * t + d if d < 7 else 84 + t
                      for t in range(T) for d in range(8)])
    wf96 = final_weight[:, f_ref].T  # [96 (8t+d), 96 (out)]
    wf = np.ascontiguousarray(
        wf96.reshape(T, 8, DIM).transpose(1, 0, 2).reshape(8, T * DIM))
    # wcomb [8, T*104]: per t, cols 0:96 = wf block(t), cols 96:104 = w2s(t)
    wcomb = np.zeros((8, T, 104), np.float32)
    for t in range(T):
        wcomb[:, t, 0:DIM] = wf[:, t * DIM:(t + 1) * DIM]
        wcomb[:, t, DIM:104] = w2[:, t * 8:(t + 1) * 8]
    wcombm = np.zeros((40, T, 104), np.float32)
    for t in range(T):
        tau = t % 4
        wcombm[8 * tau:8 * tau + 8, t, :] = wcomb[:, t, :]
        wcombm[32:40, t, :] = wcomb[:, t, :]
    wcombm = np.ascontiguousarray(wcombm.reshape(40, T * 104))

    in_maps = []
    for c in range(NCORES):
        c0 = c * NPC
        ac = np.zeros((NK, NP), np.float32)
        ac[:N, :NPC] = adjT[:, c0:c0 + NPC]
        # a[p, g, i, n] = ac[(2g+i)*KT + p, n]
        ah = np.ascontiguousarray(
            ac.reshape(NKP, 2, KT, NP).transpose(2, 0, 1, 3)).astype(FP8_NP)
        rbi = np.zeros((1, NP), np.float32)
        rbi[0, :NPC] = inv_deg[c0:c0 + NPC]
        dt96 = np.zeros((DIM, NP), np.float32)
        pt = np.zeros((8, T, NP), np.float32)
        for t in range(T):
            dt96[8 * t:8 * t + 8, :NPC] = data[t, c0:c0 + NPC, :].T
        pt[:, :, :NPC] = pos[:, c0:c0 + NPC, :].transpose(2, 0, 1)
        wbig96 = np.concatenate([w1r96, w1a96, dt96], axis=1)
        in_maps.append({
            "a": ah, "xh": xh_h,
            "rbi": rbi.astype(BF16_NP),
            "wbig96": wbig96.astype(BF16_NP),
            "pt": pt.astype(BF16_NP),
            "w1p96": w1p96.astype(BF16_NP),
            "wcombm": wcombm.astype(BF16_NP),
        })
    return in_maps


def assemble(results):
    out = np.empty((N, DIM), np.float32)
    for c in range(NCORES):
        out[c * NPC:(c + 1) * NPC, :] = \
            results[c]["out"][:, :NPC].T.astype(np.float32)
    return out


_NC_CACHE = None


def get_nc():
    global _NC_CACHE
    if _NC_CACHE is None:
        _NC_CACHE = build_nc()
    return _NC_CACHE


def run_spmd(in_maps, **kwargs):
    nc = get_nc()
    return bass_utils.run_bass_kernel_spmd(
        nc, in_maps, list(range(NCORES)), **kwargs)


def kernel(**inputs):
    in_maps = prep_in_maps(**inputs)
    res = run_spmd(in_maps)
    return assemble(res.results)


# revision 23
# speedup vs baseline: 1.0613x; 1.0613x over previous
"""Trainium2 Bass kernel for nn_CombinedGNN (gnn_message_passing).

Strategy (8 NeuronCores, node/row parallel, zero collectives):
  - masks[1] in the reference is identically zero (elementwise pow of a 0/1
    matrix), so only mask0 = adj/rowdeg matters.
  - All T=12 timesteps' aggregations are mask0 @ data[t] -> batched into ONE
    accumulated matmul  X^T @ adjT  with X = data rearranged to [N, 96].
    The row 1/deg scaling uses a host-precomputed inverse-degree vector
    broadcast to 96 partitions via a tiny matmul.
  - adj is exactly representable in fp8e4 (0/1): the big matmul runs in
    fp8 DoubleRow perf mode. X is quantized to fp8e4 hi-only (measured
    end-to-end rel err ~6e-3, well under the 2e-2 gate).
  - Per-pair (256 contraction rows) DMA granularity on both HWDGE rings
    with a greedy rate-balanced schedule so phase 1 chases the stream.
  - Dummy warm-up matmuls run from kernel start so the PE HAM clock-gate
    unthrottles (1.2 -> 2.4 GHz) before phase 1, and PE-filler work keeps
    it warm through the chain.
  - The sequential t-chain runs [feature-on-partition, node-on-free].
    Serial path per t is only: relu(prevacc) -> prev-matmul -> relu(p1)
    -> pcomb-matmul.  The raw/agg partial matmuls (batched 4 timesteps
    per 32-aligned PSUM window, PE operands must be 32-aligned) and the
    pos contribution (folded into the pcomb accumulation as extra
    matmuls: W@(relu(p1)+pos) = W@relu(p1) + W@pos) are off-path.
    Per-t prev matmuls write their 4-t window with zero-padded weight
    columns so the PSUM out stays 32-aligned.
"""

import numpy as np
import ml_dtypes

import concourse.bass as bass
import concourse.mybir as mybir
import concourse.bass_utils as bass_utils
from concourse.tile import TileContext
from concourse.vector_clock import ScopedClock
from contextlib import contextmanager


@contextmanager
def _lean_drain():
    """Skip end-of-kernel semaphore clears (one-shot NEFF; every
    run_bass_kernel_spmd call reloads the NEFF, which re-zeros sems)."""
    orig = TileContext._drain_and_barrier

    def patched(self, tick_clock, wait_clock):
        nc = self.nc
        drain_inst = nc.sync.drain()
        wait_clock.add_sem_waits(
            drain_inst.ins, ScopedClock({None: tick_clock.global_clock}))
        popped = nc._tile_sem_poison_stack.pop()
        assert popped is self._sem_poison

    TileContext._drain_and_barrier = patched
    try:
        yield
    finally:
        TileContext._drain_and_barrier = orig

# problem constants (hardcoded per harness contract)
N, T, DAY, L = 5000, 12, 8, 2
F = DAY - 1
DIM = T * DAY  # 96
NCORES = 8
NPC = N // NCORES        # 625 nodes per core
NP = 640                 # padded nodes per core
NH = NP // 2             # 320, node half processed per psum chunk
KT = 128                 # contraction tile (partitions; K padded to 5120)
NK = 5120                # padded contraction size
NKT = NK // KT           # 40 k-tiles
NKP = NKT // 2           # 20 DoubleRow k-pairs
XW = DIM                 # 96
NB = 3                   # 4-timestep blocks (32 p1 rows each)

F32 = mybir.dt.float32
BF16 = mybir.dt.bfloat16
FP8 = mybir.dt.float8e4
BF16_NP = ml_dtypes.bfloat16
FP8_NP = ml_dtypes.float8_e4m3
DR = mybir.MatmulPerfMode.DoubleRow
RELU = mybir.ActivationFunctionType.Relu

_MAXW = 1


def split_multi_waits(nc):
    """Walrus in this container rejects instructions with >~2 sync waits.
    Hoist extra waits onto preceding single-wait NoOps on the same engine."""
    f = nc.m.functions[0]
    for bb in list(f.blocks):
        new, ctr = [], 0
        for inst in bb.instructions:
            si = inst.sync_info
            waits = list(si.on_wait) if (si and si.on_wait) else []
            if len(waits) > _MAXW:
                head, keep = waits[:-_MAXW], waits[-_MAXW:]
                for i in range(0, len(head), _MAXW):
                    nop = mybir.InstNoOp(
                        name=f"{inst.name}-wsplit{ctr}", engine=inst.engine,
                        ins=[], outs=[],
                        sync_info=mybir.SyncInfo(on_wait=head[i:i + _MAXW],
                                                 on_update=[]),
                    )
                    ctr += 1
                    new.append(nop)
                inst.sync_info = mybir.SyncInfo(
                    on_wait=keep,
                    on_update=list(si.on_update) if si.on_update else [])
            new.append(inst)
        bb.instructions = new


def build_nc():
    with _lean_drain():
        return _build_nc_inner()


def _build_nc_inner():
    nc = bass.Bass()
    # a[p, g, i, n] = adjT[(2g+i)*KT + p, n]  (fp8, 0/1 exact)
    a_d = nc.dram_tensor("a", [KT, NKP, 2, NP], FP8, kind="ExternalInput")
    # xh[p, g, i, c] = Xhi[(2g+i)*KT + p, c]
    xh_d = nc.dram_tensor("xh", [KT, NKP, 2, XW], FP8, kind="ExternalInput")
    rbi_d = nc.dram_tensor("rbi", [1, NP], BF16, kind="ExternalInput")
    # wbig96 [96, 832]: cols 0:96 w1r96, 96:192 w1a96, 192:832 dt96
    # (one DMA; block-diagonal raw/agg weights + dt96[8t+f, n] staging)
    wbig96_d = nc.dram_tensor("wbig96", [DIM, 832], BF16,
                              kind="ExternalInput")
    # pt[f, t, n] = pos[t, n, f]
    pt_d = nc.dram_tensor("pt", [8, T, NP], BF16, kind="ExternalInput")
    # w1p96[p, 96t + 8t+o] = prev weight (t) at cols 8t:8t+8, rest zero
    w1p96_d = nc.dram_tensor("w1p96", [8, T * DIM], BF16,
                             kind="ExternalInput")
    # wcombm [40, T, 104]: rows 0:32 = wcomb weights at 8*(t%4)+d (the
    # relu'd 32-row h2 window; zero rows mask the window's incomplete
    # sibling rows), rows 32:40 = the same weights applied to pos(t) (pt
    # is DMA'd into h2 rows 32:40 once) -- pos folds into the SAME matmul
    wcombm_d = nc.dram_tensor("wcombm", [40, T * 104], BF16,
                              kind="ExternalInput")
    out_d = nc.dram_tensor("out", [DIM, NP], BF16, kind="ExternalOutput")

    with TileContext(nc) as tc:
        with (
            tc.tile_pool(name="const", bufs=1) as cpool,
            tc.tile_pool(name="pagg", bufs=2, space="PSUM") as pagg,
            tc.tile_pool(name="prbp1", bufs=2, space="PSUM") as prbp1,
            tc.tile_pool(name="ppc", bufs=2, space="PSUM") as ppc,
            tc.tile_pool(name="pwarm", bufs=1, space="PSUM") as pwarm,
        ):
            # ---------- SBUF tiles ----------
            warm_l = cpool.tile([1, 64], BF16)
            warm_r = cpool.tile([1, 256], BF16)
            ones_t = cpool.tile([1, DIM], BF16)
            rbi_t = cpool.tile([1, NP], BF16)
            xh_t = cpool.tile([KT, NKP, 2, XW], FP8)
            a_t = cpool.tile([KT, NKP, 2, NP], FP8)
            wbig96_t = cpool.tile([DIM, 832], BF16)
            w1p96_t = cpool.tile([8, T * DIM], BF16)
            wcombm_t = cpool.tile([40, T * 104], BF16)
            dagp_t = cpool.tile([8, T, NP], BF16)   # prev staging (t>=1)
            aggs_t = cpool.tile([DIM, NP], BF16)    # scaled agg
            rbs_t = cpool.tile([DIM, NP], BF16)     # inv-degree broadcast
            # rows 0:32 relu(p1) windows, rows 32:40 pos (DMA'd once)
            h2_t = cpool.tile([40, T, NP], BF16)
            outt_t = cpool.tile([DIM, NP], BF16)

            # ---------- DMA schedule ----------
            # Two HWDGE rings, few big transfers (each dma_start costs
            # ~650ns of engine issue + a completion-sem lane; narrow or
            # small transfers starve a ring when they lead it; a 3rd
            # (SWDGE) queue lowers aggregate throughput even when it
            # carries critical bytes).
            nc.sync.dma_start(out=rbi_t, in_=rbi_d[:, :])
            nc.sync.dma_start(out=xh_t[:, 0:10, :, :],
                              in_=xh_d[:, 0:10, :, :])
            nc.sync.dma_start(out=a_t[:, 0:5, :, :], in_=a_d[:, 0:5, :, :])
            nc.scalar.dma_start(out=a_t[:, 5:9, :, :],
                                in_=a_d[:, 5:9, :, :])
            nc.sync.dma_start(out=xh_t[:, 10:NKP, :, :],
                              in_=xh_d[:, 10:NKP, :, :])
            nc.sync.dma_start(out=a_t[:, 9:14, :, :],
                              in_=a_d[:, 9:14, :, :])
            nc.scalar.dma_start(out=a_t[:, 14:NKP, :, :],
                                in_=a_d[:, 14:NKP, :, :])
            nc.sync.dma_start(out=wbig96_t, in_=wbig96_d[:, :])
            nc.sync.dma_start(out=w1p96_t, in_=w1p96_d[:, :])
            nc.scalar.dma_start(out=h2_t[32:40, :, :], in_=pt_d[:, :, :])
            nc.scalar.dma_start(out=wcombm_t, in_=wcombm_d[:, :])

            # ---------- PSUM tiles ----------
            aggp_t = [pagg.tile([XW, NH], F32, tag="aggp", name=f"aggp{h}")
                      for h in range(2)]
            rb_t = [prbp1.tile([DIM, NH], F32, tag="rbp1", name=f"rb{h}")
                    for h in range(2)]
            warmp = pwarm.tile([64, 256], F32, tag="warm", name="warmp")

            # ---------- warm-up / filler matmuls ----------
            nc.vector.memset(warm_l, 0.125)
            nc.vector.memset(warm_r, 0.125)
            nc.vector.memset(ones_t, 1.0)

            def filler(n):
                for _ in range(n):
                    nc.tensor.matmul(warmp, warm_l, warm_r,
                                     start=True, stop=True,
                                     skip_group_check=True)

            filler(12)

            # inv-degree broadcast to 96 partitions, then to SBUF via DVE
            for h in range(2):
                nc.tensor.matmul(rb_t[h], ones_t,
                                 rbi_t[:, h * NH:(h + 1) * NH],
                                 start=True, stop=True, skip_group_check=True)
            for h in range(2):
                nc.vector.tensor_scalar_max(
                    rbs_t[:, h * NH:(h + 1) * NH], rb_t[h], 0.0)

            # p1 tiles (recycle the rb PSUM slots): rows 8t:8t+8 = step t
            p1_t = [prbp1.tile([DIM, NH], F32, tag="rbp1", name=f"p1_{h}")
                    for h in range(2)]
            pcomb_t = [ppc.tile([104, NH], F32, tag="pcm", name=f"pcomb{h}")
                       for h in range(2)]

            # ---------- phase 2 helpers ----------
            def partial_raw(h):
                cs = slice(192 + h * NH, 192 + (h + 1) * NH)
                nc.tensor.matmul(p1_t[h], wbig96_t[:, 0:DIM],
                                 wbig96_t[:, cs],
                                 start=True, stop=False,
                                 skip_group_check=True)

            def partial_agg(h):
                cs = slice(h * NH, (h + 1) * NH)
                nc.tensor.matmul(p1_t[h], wbig96_t[:, DIM:2 * DIM],
                                 aggs_t[:, cs],
                                 start=False, stop=False,
                                 skip_group_check=True)

            def prev_mms(t):
                # adds W_p^T @ prev into rows 8t:8t+8 (weight cols outside
                # the step's 8 rows are zero, so the other 88 rows
                # accumulate +0; keeps all operands at partition base 0)
                for h in range(2):
                    cs = slice(h * NH, (h + 1) * NH)
                    nc.tensor.matmul(p1_t[h],
                                     w1p96_t[:, DIM * t:DIM * (t + 1)],
                                     dagp_t[0:8, t, cs],
                                     start=False,
                                     stop=(t == T - 1),
                                     skip_group_check=True)

            def relu_p1(t):
                # PSUM partition base must be 32-aligned: relu the whole
                # 32-row window (same cost, DVE/ACT time is free-dim-bound);
                # sibling rows may hold partial sums - wcomb4's zero rows
                # mask them in the pcomb matmul.
                b = t // 4
                w = slice(32 * b, 32 * b + 32)
                nc.scalar.activation(h2_t[0:32, t, 0:NH],
                                     p1_t[0][w, :], RELU)
                nc.vector.tensor_scalar_max(h2_t[0:32, t, NH:NP],
                                            p1_t[1][w, :], 0.0)

            def pcomb_mms(t):
                # one matmul adds BOTH wcomb^T relu(p1) and wcomb^T pos(t)
                wc = slice(t * 104, (t + 1) * 104)
                for h in range(2):
                    cs = slice(h * NH, (h + 1) * NH)
                    nc.tensor.matmul(pcomb_t[h], wcombm_t[:, wc],
                                     h2_t[:, t, cs],
                                     start=(t == 0), stop=(t == T - 1),
                                     skip_group_check=True)

            def relu_prev(t):
                # prevacc rows 96:104, mid-group read -> dagp[:, t+1]
                nc.vector.tensor_scalar_max(
                    dagp_t[0:8, t + 1, 0:NH], pcomb_t[0][DIM:104, :], 0.0)
                nc.scalar.activation(dagp_t[0:8, t + 1, NH:NP],
                                     pcomb_t[1][DIM:104, :], RELU)

            # ---------- phase 1: aggT[96, NH] += Xhi^T @ adjT, fp8 DR ------
            # pair order = expected DMA arrival order across the two rings
            # consume a-pairs in DMA-arrival order (the accumulation is
            # order-agnostic): scalar's slices a[5:9]/a[15:20] land
            # ~11-16us while sync's land ~18-22us
            PERM = [5, 6, 7, 8, 14, 15, 16, 17, 18, 19,
                    0, 1, 2, 3, 4, 9, 10, 11, 12, 13]
            for i, g in enumerate(PERM):
                filler(1)
                for h in range(2):
                    nc.tensor.matmul(
                        aggp_t[h], xh_t[:, g, :, :],
                        a_t[:, g, :, h * NH:(h + 1) * NH],
                        start=(i == 0), stop=(i == NKP - 1),
                        perf_mode=DR, skip_group_check=True)

            # ---------- transition: scale by inv-degree ----------
            def scale(h):
                cs = slice(h * NH, (h + 1) * NH)
                nc.vector.tensor_mul(aggs_t[:, cs], aggp_t[h],
                                     rbs_t[:, cs])

            # ---------- phase 2: issue order ----------
            # raw partials + fillers bridge the PE through the scale ops
            # so the HAM clock-gate stays unthrottled into the chain;
            # chain fillers keep PE density high enough to hold it there.
            partial_raw(0)
            partial_raw(1)
            scale(0)
            scale(1)
            partial_agg(0)
            partial_agg(1)
            relu_p1(0)
            pcomb_mms(0)
            relu_prev(0)
            for t in range(1, T):
                prev_mms(t)
                filler(2)
                relu_p1(t)
                pcomb_mms(t)
                if t < T - 1:
                    relu_prev(t)

            # ---------- final relu + store ----------
            nc.scalar.activation(outt_t[:, 0:NH], pcomb_t[0][0:DIM, :], RELU)
            nc.vector.tensor_scalar_max(outt_t[:, NH:NP],
                                        pcomb_t[1][0:DIM, :], 0.0)
            nc.sync.dma_start(out=out_d[:, 0:NH], in_=outt_t[:, 0:NH])
            nc.scalar.dma_start(out=out_d[:, NH:NP], in_=outt_t[:, NH:NP])

    split_multi_waits(nc)
    return nc


def prep_in_maps(adj, data, pos, his_W, cur_W, his_weight, cur_weight,
                 final_weight):
    adj = np.asarray(adj, dtype=np.float32)
    data = np.asarray(data, dtype=np.float32)
    pos = np.asarray(pos, dtype=np.float32)
    his_W = np.asarray(his_W, dtype=np.float32)
    cur_W = np.asarray(cur_W, dtype=np.float32)
    his_weight = np.asarray(his_weight, dtype=np.float32)
    cur_weight = np.asarray(cur_weight, dtype=np.float32)
    final_weight = np.asarray(final_weight, dtype=np.float32)

    # X = data rearranged [N, 96] (col = t*8+d); zero-padded to NK rows;
    # fp8e4 hi quantization (lo pass dropped; error budget allows).
    X = np.ascontiguousarray(data.transpose(1, 0, 2).reshape(N, DIM))
    Xe = np.zeros((NK, XW), np.float32)
    Xe[:N, :] = X
    Xhi = Xe.astype(FP8_NP)
    # DoubleRow lhsT layout: [KT, NKP, 2, XW], [p, g, i, c] = row (2g+i)*KT+p
    def dr_pack(M):
        return np.ascontiguousarray(
            M.reshape(NKP, 2, KT, M.shape[1]).transpose(2, 0, 1, 3))
    xh_h = dr_pack(Xhi)

    adjT = np.ascontiguousarray(adj.T)
    inv_deg = 1.0 / np.maximum(adj.sum(axis=1), 1.0)  # [N]

    # w1 [24, 96]: rows 0:8 prev-block, 8:16 raw, 16:24 agg  (per t cols)
    w1 = np.zeros((24, DIM), np.float32)
    for t in range(T):
        w1[0:7, t * 8:t * 8 + 7] = his_W[t][:, 21:28].T
        w1[7, t * 8 + 7] = cur_W[t][0, 3]
        w1[8:15, t * 8:t * 8 + 7] = his_W[t][:, 0:7].T
        w1[15, t * 8 + 7] = cur_W[t][0, 0]
        w1[16:23, t * 8:t * 8 + 7] = his_W[t][:, 7:14].T
        w1[23, t * 8 + 7] = cur_W[t][0, 1]
    # block-diagonal 96-row lhsT packs (all operands at partition base 0)
    w1r96 = np.zeros((DIM, DIM), np.float32)
    w1a96 = np.zeros((DIM, DIM), np.float32)
    w1p96 = np.zeros((8, T, DIM), np.float32)
    for t in range(T):
        s = slice(8 * t, 8 * t + 8)
        w1r96[s, s] = w1[8:16, s]
        w1a96[s, s] = w1[16:24, s]
        w1p96[:, t, s] = w1[0:8, s]
    w1p96 = np.ascontiguousarray(w1p96.reshape(8, T * DIM))

    # w2s[d, 8t'+o] = prev-update weight from h(t') feature d to output o
    w2 = np.zeros((8, DIM), np.float32)
    for tp in range(T):
        w2[0:7, tp * 8:tp * 8 + 7] = his_weight[:, 7 * tp:7 * tp + 7].T
        w2[7, tp * 8 + 7] = cur_weight[0, tp]
    # interleaved feature (8t+d) -> reference feature (7t+d | 84+t)
    f_ref = np.array([7 * t + d if d < 7 else 84 + t
                      for t in range(T) for d in range(8)])
    wf96 = final_weight[:, f_ref].T  # [96 (8t+d), 96 (out)]
    wf = np.ascontiguousarray(
        wf96.reshape(T, 8, DIM).transpose(1, 0, 2).reshape(8, T * DIM))
    # wcomb [8, T*104]: per t, cols 0:96 = wf block(t), cols 96:104 = w2s(t)
    wcomb = np.zeros((8, T, 104), np.float32)
    for t in range(T):
        wcomb[:, t, 0:DIM] = wf[:, t * DIM:(t + 1) * DIM]
        wcomb[:, t, DIM:104] = w2[:, t * 8:(t + 1) * 8]
    wcombm = np.zeros((40, T, 104), np.float32)
    for t in range(T):
        tau = t % 4
        wcombm[8 * tau:8 * tau + 8, t, :] = wcomb[:, t, :]
        wcombm[32:40, t, :] = wcomb[:, t, :]
    wcombm = np.ascontiguousarray(wcombm.reshape(40, T * 104))

    in_maps = []
    for c in range(NCORES):
        c0 = c * NPC
        ac = np.zeros((NK, NP), np.float32)
        ac[:N, :NPC] = adjT[:, c0:c0 + NPC]
        # a[p, g, i, n] = ac[(2g+i)*KT + p, n]
        ah = np.ascontiguousarray(
            ac.reshape(NKP, 2, KT, NP).transpose(2, 0, 1, 3)).astype(FP8_NP)
        rbi = np.zeros((1, NP), np.float32)
        rbi[0, :NPC] = inv_deg[c0:c0 + NPC]
        dt96 = np.zeros((DIM, NP), np.float32)
        pt = np.zeros((8, T, NP), np.float32)
        for t in range(T):
            dt96[8 * t:8 * t + 8, :NPC] = data[t, c0:c0 + NPC, :].T
        pt[:, :, :NPC] = pos[:, c0:c0 + NPC, :].transpose(2, 0, 1)
        wbig96 = np.concatenate([w1r96, w1a96, dt96], axis=1)
        in_maps.append({
            "a": ah, "xh": xh_h,
            "rbi": rbi.astype(BF16_NP),
            "wbig96": wbig96.astype(BF16_NP),
            "pt": pt.astype(BF16_NP),
            "w1p96": w1p96.astype(BF16_NP),
            "wcombm": wcombm.astype(BF16_NP),
        })
    return in_maps


def assemble(results):
    out = np.empty((N, DIM), np.float32)
    for c in range(NCORES):
        out[c * NPC:(c + 1) * NPC, :] = \
            results[c]["out"][:, :NPC].T.astype(np.float32)
    return out


_NC_CACHE = None


def get_nc():
    global _NC_CACHE
    if _NC_CACHE is None:
        _NC_CACHE = build_nc()
    return _NC_CACHE


def run_spmd(in_maps, **kwargs):
    nc = get_nc()
    return bass_utils.run_bass_kernel_spmd(
        nc, in_maps, list(range(NCORES)), **kwargs)


def kernel(**inputs):
    in_maps = prep_in_maps(**inputs)
    res = run_spmd(in_maps)
    return assemble(res.results)


# revision 24
# speedup vs baseline: 1.0836x; 1.0211x over previous
"""Trainium2 Bass kernel for nn_CombinedGNN (gnn_message_passing).

Strategy (8 NeuronCores, node/row parallel, zero collectives):
  - masks[1] in the reference is identically zero (elementwise pow of a 0/1
    matrix), so only mask0 = adj/rowdeg matters.
  - All T=12 timesteps' aggregations are mask0 @ data[t] -> batched into ONE
    accumulated matmul  X^T @ adjT  with X = data rearranged to [N, 96].
    The row 1/deg scaling uses a host-precomputed inverse-degree vector
    broadcast to 96 partitions via a tiny matmul.
  - adj is exactly representable in fp8e4 (0/1): the big matmul runs in
    fp8 DoubleRow perf mode. X is quantized to fp8e4 hi-only (measured
    end-to-end rel err ~6e-3, well under the 2e-2 gate).
  - Per-pair (256 contraction rows) DMA granularity on both HWDGE rings
    with a greedy rate-balanced schedule so phase 1 chases the stream.
  - Dummy warm-up matmuls run from kernel start so the PE HAM clock-gate
    unthrottles (1.2 -> 2.4 GHz) before phase 1, and PE-filler work keeps
    it warm through the chain.
  - The sequential t-chain runs [feature-on-partition, node-on-free].
    Serial path per t is only: relu(prevacc) -> prev-matmul -> relu(p1)
    -> pcomb-matmul.  The raw/agg partial matmuls (batched 4 timesteps
    per 32-aligned PSUM window, PE operands must be 32-aligned) and the
    pos contribution (folded into the pcomb accumulation as extra
    matmuls: W@(relu(p1)+pos) = W@relu(p1) + W@pos) are off-path.
    Per-t prev matmuls write their 4-t window with zero-padded weight
    columns so the PSUM out stays 32-aligned.
"""

import numpy as np
import ml_dtypes

import concourse.bass as bass
import concourse.mybir as mybir
import concourse.bass_utils as bass_utils
from concourse.tile import TileContext
from concourse.vector_clock import ScopedClock
from contextlib import contextmanager


@contextmanager
def _lean_drain():
    """Skip end-of-kernel semaphore clears (one-shot NEFF; every
    run_bass_kernel_spmd call reloads the NEFF, which re-zeros sems)."""
    orig = TileContext._drain_and_barrier

    def patched(self, tick_clock, wait_clock):
        nc = self.nc
        drain_inst = nc.sync.drain()
        wait_clock.add_sem_waits(
            drain_inst.ins, ScopedClock({None: tick_clock.global_clock}))
        popped = nc._tile_sem_poison_stack.pop()
        assert popped is self._sem_poison

    TileContext._drain_and_barrier = patched
    try:
        yield
    finally:
        TileContext._drain_and_barrier = orig

# problem constants (hardcoded per harness contract)
N, T, DAY, L = 5000, 12, 8, 2
F = DAY - 1
DIM = T * DAY  # 96
NCORES = 8
NPC = N // NCORES        # 625 nodes per core
NP = 640                 # padded nodes per core
NH = NP // 2             # 320, node half processed per psum chunk
KT = 128                 # contraction tile (partitions; K padded to 5120)
NK = 5120                # padded contraction size
NKT = NK // KT           # 40 k-tiles
NKP = NKT // 2           # 20 DoubleRow k-pairs
XW = DIM                 # 96
NB = 3                   # 4-timestep blocks (32 p1 rows each)

F32 = mybir.dt.float32
BF16 = mybir.dt.bfloat16
FP8 = mybir.dt.float8e4
BF16_NP = ml_dtypes.bfloat16
FP8_NP = ml_dtypes.float8_e4m3
DR = mybir.MatmulPerfMode.DoubleRow
RELU = mybir.ActivationFunctionType.Relu

_MAXW = 1


def split_multi_waits(nc):
    """Walrus in this container rejects instructions with >~2 sync waits.
    Hoist extra waits onto preceding single-wait NoOps on the same engine."""
    f = nc.m.functions[0]
    for bb in list(f.blocks):
        new, ctr = [], 0
        for inst in bb.instructions:
            si = inst.sync_info
            waits = list(si.on_wait) if (si and si.on_wait) else []
            if len(waits) > _MAXW:
                head, keep = waits[:-_MAXW], waits[-_MAXW:]
                for i in range(0, len(head), _MAXW):
                    nop = mybir.InstNoOp(
                        name=f"{inst.name}-wsplit{ctr}", engine=inst.engine,
                        ins=[], outs=[],
                        sync_info=mybir.SyncInfo(on_wait=head[i:i + _MAXW],
                                                 on_update=[]),
                    )
                    ctr += 1
                    new.append(nop)
                inst.sync_info = mybir.SyncInfo(
                    on_wait=keep,
                    on_update=list(si.on_update) if si.on_update else [])
            new.append(inst)
        bb.instructions = new


def build_nc():
    with _lean_drain():
        return _build_nc_inner()


def _build_nc_inner():
    nc = bass.Bass()
    # ax[p, g, i, 0:NP] = adjT[(2g+i)*KT+p, n]; ax[p, g, i, NP:NP+96] =
    # Xhi[(2g+i)*KT+p, c]  (fp8; one tensor so each pair's stationary and
    # moving operands arrive in the same transfer)
    ax_d = nc.dram_tensor("ax", [KT, NKP, 2, NP + XW], FP8,
                          kind="ExternalInput")
    rbi_d = nc.dram_tensor("rbi", [1, NP], BF16, kind="ExternalInput")
    # wbig96 [96, 832]: cols 0:96 w1r96, 96:192 w1a96, 192:832 dt96
    # (one DMA; block-diagonal raw/agg weights + dt96[8t+f, n] staging)
    wbig96_d = nc.dram_tensor("wbig96", [DIM, 832], BF16,
                              kind="ExternalInput")
    # pt[f, t, n] = pos[t, n, f]
    pt_d = nc.dram_tensor("pt", [8, T, NP], BF16, kind="ExternalInput")
    # w1p96[p, 96t + 8t+o] = prev weight (t) at cols 8t:8t+8, rest zero
    w1p96_d = nc.dram_tensor("w1p96", [8, T * DIM], BF16,
                             kind="ExternalInput")
    # wcombm [40, T, 104]: rows 0:32 = wcomb weights at 8*(t%4)+d (the
    # relu'd 32-row h2 window; zero rows mask the window's incomplete
    # sibling rows), rows 32:40 = the same weights applied to pos(t) (pt
    # is DMA'd into h2 rows 32:40 once) -- pos folds into the SAME matmul
    wcombm_d = nc.dram_tensor("wcombm", [40, T * 104], BF16,
                              kind="ExternalInput")
    out_d = nc.dram_tensor("out", [DIM, NP], BF16, kind="ExternalOutput")

    with TileContext(nc) as tc:
        with (
            tc.tile_pool(name="const", bufs=1) as cpool,
            tc.tile_pool(name="pagg", bufs=2, space="PSUM") as pagg,
            tc.tile_pool(name="prbp1", bufs=2, space="PSUM") as prbp1,
            tc.tile_pool(name="ppc", bufs=2, space="PSUM") as ppc,
            tc.tile_pool(name="pwarm", bufs=1, space="PSUM") as pwarm,
        ):
            # ---------- SBUF tiles ----------
            warm_l = cpool.tile([1, 64], BF16)
            warm_r = cpool.tile([1, 256], BF16)
            ones_t = cpool.tile([1, DIM], BF16)
            rbi_t = cpool.tile([1, NP], BF16)
            ax_t = cpool.tile([KT, NKP, 2, NP + XW], FP8)
            wbig96_t = cpool.tile([DIM, 832], BF16)
            w1p96_t = cpool.tile([8, T * DIM], BF16)
            wcombm_t = cpool.tile([40, T * 104], BF16)
            dagp_t = cpool.tile([8, T, NP], BF16)   # prev staging (t>=1)
            aggs_t = cpool.tile([DIM, NP], BF16)    # scaled agg
            rbs_t = cpool.tile([DIM, NP], BF16)     # inv-degree broadcast
            # rows 0:32 relu(p1) windows, rows 32:40 pos (DMA'd once)
            h2_t = cpool.tile([40, T, NP], BF16)
            outt_t = cpool.tile([DIM, NP], BF16)

            # ---------- DMA schedule ----------
            # Two HWDGE rings, few big transfers (each dma_start costs
            # ~650ns of engine issue + a completion-sem lane; narrow or
            # small transfers starve a ring when they lead it; a 3rd
            # (SWDGE) queue lowers aggregate throughput even when it
            # carries critical bytes).
            nc.sync.dma_start(out=rbi_t, in_=rbi_d[:, :])
            nc.sync.dma_start(out=ax_t[:, 0:5, :, :],
                              in_=ax_d[:, 0:5, :, :])
            nc.scalar.dma_start(out=ax_t[:, 5:9, :, :],
                                in_=ax_d[:, 5:9, :, :])
            nc.sync.dma_start(out=ax_t[:, 9:15, :, :],
                              in_=ax_d[:, 9:15, :, :])
            nc.scalar.dma_start(out=ax_t[:, 15:NKP, :, :],
                                in_=ax_d[:, 15:NKP, :, :])
            nc.sync.dma_start(out=wbig96_t, in_=wbig96_d[:, :])
            nc.sync.dma_start(out=w1p96_t, in_=w1p96_d[:, :])
            nc.scalar.dma_start(out=h2_t[32:40, :, :], in_=pt_d[:, :, :])
            nc.scalar.dma_start(out=wcombm_t, in_=wcombm_d[:, :])

            # ---------- PSUM tiles ----------
            aggp_t = [pagg.tile([XW, NH], F32, tag="aggp", name=f"aggp{h}")
                      for h in range(2)]
            rb_t = [prbp1.tile([DIM, NH], F32, tag="rbp1", name=f"rb{h}")
                    for h in range(2)]
            warmp = pwarm.tile([64, 256], F32, tag="warm", name="warmp")

            # ---------- warm-up / filler matmuls ----------
            nc.vector.memset(warm_l, 0.125)
            nc.vector.memset(warm_r, 0.125)
            nc.vector.memset(ones_t, 1.0)

            def filler(n):
                for _ in range(n):
                    nc.tensor.matmul(warmp, warm_l, warm_r,
                                     start=True, stop=True,
                                     skip_group_check=True)

            filler(12)

            # inv-degree broadcast to 96 partitions, then to SBUF via DVE
            for h in range(2):
                nc.tensor.matmul(rb_t[h], ones_t,
                                 rbi_t[:, h * NH:(h + 1) * NH],
                                 start=True, stop=True, skip_group_check=True)
            for h in range(2):
                nc.vector.tensor_scalar_max(
                    rbs_t[:, h * NH:(h + 1) * NH], rb_t[h], 0.0)

            # p1 tiles (recycle the rb PSUM slots): rows 8t:8t+8 = step t
            p1_t = [prbp1.tile([DIM, NH], F32, tag="rbp1", name=f"p1_{h}")
                    for h in range(2)]
            pcomb_t = [ppc.tile([104, NH], F32, tag="pcm", name=f"pcomb{h}")
                       for h in range(2)]

            # ---------- phase 2 helpers ----------
            def partial_raw(h):
                cs = slice(192 + h * NH, 192 + (h + 1) * NH)
                nc.tensor.matmul(p1_t[h], wbig96_t[:, 0:DIM],
                                 wbig96_t[:, cs],
                                 start=True, stop=False,
                                 skip_group_check=True)

            def partial_agg(h):
                cs = slice(h * NH, (h + 1) * NH)
                nc.tensor.matmul(p1_t[h], wbig96_t[:, DIM:2 * DIM],
                                 aggs_t[:, cs],
                                 start=False, stop=False,
                                 skip_group_check=True)

            def prev_mms(t):
                # adds W_p^T @ prev into rows 8t:8t+8 (weight cols outside
                # the step's 8 rows are zero, so the other 88 rows
                # accumulate +0; keeps all operands at partition base 0)
                for h in range(2):
                    cs = slice(h * NH, (h + 1) * NH)
                    nc.tensor.matmul(p1_t[h],
                                     w1p96_t[:, DIM * t:DIM * (t + 1)],
                                     dagp_t[0:8, t, cs],
                                     start=False,
                                     stop=(t == T - 1),
                                     skip_group_check=True)

            def relu_p1(t):
                # PSUM partition base must be 32-aligned: relu the whole
                # 32-row window (same cost, DVE/ACT time is free-dim-bound);
                # sibling rows may hold partial sums - wcomb4's zero rows
                # mask them in the pcomb matmul.
                b = t // 4
                w = slice(32 * b, 32 * b + 32)
                nc.scalar.activation(h2_t[0:32, t, 0:NH],
                                     p1_t[0][w, :], RELU)
                nc.vector.tensor_scalar_max(h2_t[0:32, t, NH:NP],
                                            p1_t[1][w, :], 0.0)

            def pcomb_mms(t):
                # one matmul adds BOTH wcomb^T relu(p1) and wcomb^T pos(t)
                wc = slice(t * 104, (t + 1) * 104)
                for h in range(2):
                    cs = slice(h * NH, (h + 1) * NH)
                    nc.tensor.matmul(pcomb_t[h], wcombm_t[:, wc],
                                     h2_t[:, t, cs],
                                     start=(t == 0), stop=(t == T - 1),
                                     skip_group_check=True)

            def relu_prev(t):
                # prevacc rows 96:104, mid-group read -> dagp[:, t+1]
                nc.vector.tensor_scalar_max(
                    dagp_t[0:8, t + 1, 0:NH], pcomb_t[0][DIM:104, :], 0.0)
                nc.scalar.activation(dagp_t[0:8, t + 1, NH:NP],
                                     pcomb_t[1][DIM:104, :], RELU)

            # ---------- phase 1: aggT[96, NH] += Xhi^T @ adjT, fp8 DR ------
            # pair order = expected DMA arrival order across the two rings
            # consume a-pairs in DMA-arrival order (the accumulation is
            # order-agnostic): scalar's slices a[5:9]/a[15:20] land
            # ~11-16us while sync's land ~18-22us
            PERM = [5, 6, 7, 8, 15, 16, 17, 18, 19,
                    0, 1, 2, 3, 4, 9, 10, 11, 12, 13, 14]
            for i, g in enumerate(PERM):
                filler(2 if i >= 14 else 1)
                for h in range(2):
                    nc.tensor.matmul(
                        aggp_t[h], ax_t[:, g, :, NP:NP + XW],
                        ax_t[:, g, :, h * NH:(h + 1) * NH],
                        start=(i == 0), stop=(i == NKP - 1),
                        perf_mode=DR, skip_group_check=True)

            # ---------- transition: scale by inv-degree ----------
            def scale(h):
                cs = slice(h * NH, (h + 1) * NH)
                nc.vector.tensor_mul(aggs_t[:, cs], aggp_t[h],
                                     rbs_t[:, cs])

            # ---------- phase 2: issue order ----------
            # raw partials + fillers bridge the PE through the scale ops
            # so the HAM clock-gate stays unthrottled into the chain;
            # chain fillers keep PE density high enough to hold it there.
            partial_raw(0)
            partial_raw(1)
            scale(0)
            scale(1)
            partial_agg(0)
            partial_agg(1)
            relu_p1(0)
            pcomb_mms(0)
            relu_prev(0)
            for t in range(1, T):
                prev_mms(t)
                filler(2)
                relu_p1(t)
                pcomb_mms(t)
                if t < T - 1:
                    relu_prev(t)

            # ---------- final relu + store ----------
            nc.scalar.activation(outt_t[:, 0:NH], pcomb_t[0][0:DIM, :], RELU)
            nc.vector.tensor_scalar_max(outt_t[:, NH:NP],
                                        pcomb_t[1][0:DIM, :], 0.0)
            nc.sync.dma_start(out=out_d[:, 0:NH], in_=outt_t[:, 0:NH])
            nc.scalar.dma_start(out=out_d[:, NH:NP], in_=outt_t[:, NH:NP])

    split_multi_waits(nc)
    return nc


def prep_in_maps(adj, data, pos, his_W, cur_W, his_weight, cur_weight,
                 final_weight):
    adj = np.asarray(adj, dtype=np.float32)
    data = np.asarray(data, dtype=np.float32)
    pos = np.asarray(pos, dtype=np.float32)
    his_W = np.asarray(his_W, dtype=np.float32)
    cur_W = np.asarray(cur_W, dtype=np.float32)
    his_weight = np.asarray(his_weight, dtype=np.float32)
    cur_weight = np.asarray(cur_weight, dtype=np.float32)
    final_weight = np.asarray(final_weight, dtype=np.float32)

    # X = data rearranged [N, 96] (col = t*8+d); zero-padded to NK rows;
    # fp8e4 hi quantization (lo pass dropped; error budget allows).
    X = np.ascontiguousarray(data.transpose(1, 0, 2).reshape(N, DIM))
    Xe = np.zeros((NK, XW), np.float32)
    Xe[:N, :] = X
    Xhi = Xe.astype(FP8_NP)
    # DoubleRow lhsT layout: [KT, NKP, 2, XW], [p, g, i, c] = row (2g+i)*KT+p
    def dr_pack(M):
        return np.ascontiguousarray(
            M.reshape(NKP, 2, KT, M.shape[1]).transpose(2, 0, 1, 3))
    xh_h = dr_pack(Xhi)

    adjT = np.ascontiguousarray(adj.T)
    inv_deg = 1.0 / np.maximum(adj.sum(axis=1), 1.0)  # [N]

    # w1 [24, 96]: rows 0:8 prev-block, 8:16 raw, 16:24 agg  (per t cols)
    w1 = np.zeros((24, DIM), np.float32)
    for t in range(T):
        w1[0:7, t * 8:t * 8 + 7] = his_W[t][:, 21:28].T
        w1[7, t * 8 + 7] = cur_W[t][0, 3]
        w1[8:15, t * 8:t * 8 + 7] = his_W[t][:, 0:7].T
        w1[15, t * 8 + 7] = cur_W[t][0, 0]
        w1[16:23, t * 8:t * 8 + 7] = his_W[t][:, 7:14].T
        w1[23, t * 8 + 7] = cur_W[t][0, 1]
    # block-diagonal 96-row lhsT packs (all operands at partition base 0)
    w1r96 = np.zeros((DIM, DIM), np.float32)
    w1a96 = np.zeros((DIM, DIM), np.float32)
    w1p96 = np.zeros((8, T, DIM), np.float32)
    for t in range(T):
        s = slice(8 * t, 8 * t + 8)
        w1r96[s, s] = w1[8:16, s]
        w1a96[s, s] = w1[16:24, s]
        w1p96[:, t, s] = w1[0:8, s]
    w1p96 = np.ascontiguousarray(w1p96.reshape(8, T * DIM))

    # w2s[d, 8t'+o] = prev-update weight from h(t') feature d to output o
    w2 = np.zeros((8, DIM), np.float32)
    for tp in range(T):
        w2[0:7, tp * 8:tp * 8 + 7] = his_weight[:, 7 * tp:7 * tp + 7].T
        w2[7, tp * 8 + 7] = cur_weight[0, tp]
    # interleaved feature (8t+d) -> reference feature (7t+d | 84+t)
    f_ref = np.array([7 * t + d if d < 7 else 84 + t
                      for t in range(T) for d in range(8)])
    wf96 = final_weight[:, f_ref].T  # [96 (8t+d), 96 (out)]
    wf = np.ascontiguousarray(
        wf96.reshape(T, 8, DIM).transpose(1, 0, 2).reshape(8, T * DIM))
    # wcomb [8, T*104]: per t, cols 0:96 = wf block(t), cols 96:104 = w2s(t)
    wcomb = np.zeros((8, T, 104), np.float32)
    for t in range(T):
        wcomb[:, t, 0:DIM] = wf[:, t * DIM:(t + 1) * DIM]
        wcomb[:, t, DIM:104] = w2[:, t * 8:(t + 1) * 8]
    wcombm = np.zeros((40, T, 104), np.float32)
    for t in range(T):
        tau = t % 4
        wcombm[8 * tau:8 * tau + 8, t, :] = wcomb[:, t, :]
        wcombm[32:40, t, :] = wcomb[:, t, :]
    wcombm = np.ascontiguousarray(wcombm.reshape(40, T * 104))

    in_maps = []
    for c in range(NCORES):
        c0 = c * NPC
        ac = np.zeros((NK, NP), np.float32)
        ac[:N, :NPC] = adjT[:, c0:c0 + NPC]
        # a[p, g, i, n] = ac[(2g+i)*KT + p, n]
        ah = np.ascontiguousarray(
            ac.reshape(NKP, 2, KT, NP).transpose(2, 0, 1, 3)).astype(FP8_NP)
        rbi = np.zeros((1, NP), np.float32)
        rbi[0, :NPC] = inv_deg[c0:c0 + NPC]
        dt96 = np.zeros((DIM, NP), np.float32)
        pt = np.zeros((8, T, NP), np.float32)
        for t in range(T):
            dt96[8 * t:8 * t + 8, :NPC] = data[t, c0:c0 + NPC, :].T
        pt[:, :, :NPC] = pos[:, c0:c0 + NPC, :].transpose(2, 0, 1)
        wbig96 = np.concatenate([w1r96, w1a96, dt96], axis=1)
        ax = np.concatenate([ah, xh_h.astype(FP8_NP)], axis=3)
        in_maps.append({
            "ax": np.ascontiguousarray(ax),
            "rbi": rbi.astype(BF16_NP),
            "wbig96": wbig96.astype(BF16_NP),
            "pt": pt.astype(BF16_NP),
            "w1p96": w1p96.astype(BF16_NP),
            "wcombm": wcombm.astype(BF16_NP),
        })
    return in_maps


def assemble(results):
    out = np.empty((N, DIM), np.float32)
    for c in range(NCORES):
        out[c * NPC:(c + 1) * NPC, :] = \
            results[c]["out"][:, :NPC].T.astype(np.float32)
    return out


_NC_CACHE = None


def get_nc():
    global _NC_CACHE
    if _NC_CACHE is None:
        _NC_CACHE = build_nc()
    return _NC_CACHE


def run_spmd(in_maps, **kwargs):
    nc = get_nc()
    return bass_utils.run_bass_kernel_spmd(
        nc, in_maps, list(range(NCORES)), **kwargs)


def kernel(**inputs):
    in_maps = prep_in_maps(**inputs)
    res = run_spmd(in_maps)
    return assemble(res.results)
